# revision 12
# baseline (speedup 1.0000x reference)
"""Cosine attention (B=1, H=16, S=4096, D=64) on 8 trn2 NeuronCores.

Sharding: batch*heads split across cores -> 2 heads per core, full seq per
head (softmax is per-row, no cross-core communication).

Per-head pipeline on a core:
  1. load Q,K,V [4096,64] in "(p t) d" layout -> [128, 32, 64]
  2. row sumsq (DVE), rsqrt via Newton iterations (DVE-only, no ACT tables),
     fold exp(min(logit_scale, ln 8)) into the Q-side scale
  3. normalize + PE-transpose to d-major: QT2 [128, 32, 128] (both partition
     halves hold the same Qt chunk), KT2 [128, 16, 128] (even chunks in
     partitions 0-63, odd in 64-127) -- rounded to float32r
  4. stage 1: row-packed fp32r matmul pairs S^T[j-chunk, i-block] -> PSUM
     slabs [128, 1536] (ping-pong)
  5. exp on ScalarE straight from PSUM -> E^T tiles (float32r)
  6. stage 2: o_ps[65, 512] += Vx[chunk].T @ E^T (Vx has a ones column so
     row 64 accumulates the softmax denominator Z)
  7. drain: copy to SBUF, PE-transpose back to row-major, multiply by
     1/Z (fast reciprocal), one 1 MB DMA out per head.
"""

import math
from contextlib import ExitStack

import numpy as np

import concourse.bass as bass
import concourse.tile as tile
from concourse import bacc, mybir
import concourse.bass_utils as bass_utils
from concourse.masks import make_identity

F32 = mybir.dt.float32
F32R = mybir.dt.float32r
I32 = mybir.dt.int32

N_CORES = 8
H_TOTAL = 16
H_PER_CORE = H_TOTAL // N_CORES
D = 64
IBW = 512          # i-block width (PSUM bank / fp32 moving-operand limit)
SLABW = 1536       # exp slab width (3 PSUM banks)


def _newton_rsqrt(nc, pool, ss, n):
    """rsqrt of ss [128, n] (fp32, positive) via bit-trick seed + 3 Newton
    iterations, all on VectorE. Returns a [128, n] fp32 AP."""
    seed_i = pool.tile([128, n], I32, tag="nw_i")
    # ~(i >> 1)
    nc.vector.tensor_scalar(
        out=seed_i[:], in0=ss.bitcast(I32), scalar1=1, scalar2=-1,
        op0=mybir.AluOpType.logical_shift_right, op1=mybir.AluOpType.bitwise_xor)
    # + (0x5f3759df + 1)  == 0x5f3759df - (i >> 1)
    nc.vector.tensor_scalar(
        out=seed_i[:], in0=seed_i[:], scalar1=0x5F3759E0, scalar2=None,
        op0=mybir.AluOpType.add)
    y = seed_i.bitcast(F32)
    t = pool.tile([128, n], F32, tag="nw_t")
    for _ in range(3):
        nc.vector.tensor_mul(t[:], y, y)            # y*y
        nc.vector.tensor_mul(t[:], t[:], ss)        # x*y*y
        nc.vector.tensor_scalar(                    # 1.5 - 0.5*x*y*y
            out=t[:], in0=t[:], scalar1=-0.5, scalar2=1.5,
            op0=mybir.AluOpType.mult, op1=mybir.AluOpType.add)
        nc.vector.tensor_mul(y, y, t[:])            # y *= t
    return y


def build_kernel(S):
    """Build the 2-head-per-core cosine attention program for seq len S."""
    NT = S // 128            # 128-row tiles per head
    NPAIR = NT // 2
    NIB = S // IBW           # i-blocks
    CPI = IBW // 128         # chunks of output columns per i-block (4)

    nc = bacc.Bacc("TRN2", target_bir_lowering=False, debug=False,
                   enable_asserts=False, num_devices=N_CORES)

    q_d = nc.dram_tensor("q", [H_PER_CORE, S, D], F32, kind="ExternalInput").ap()
    k_d = nc.dram_tensor("k", [H_PER_CORE, S, D], F32, kind="ExternalInput").ap()
    v_d = nc.dram_tensor("v", [H_PER_CORE, S, D], F32, kind="ExternalInput").ap()
    qs_d = nc.dram_tensor("qs", [128, H_PER_CORE], F32, kind="ExternalInput").ap()
    o_d = nc.dram_tensor("o", [H_PER_CORE, S, D], F32, kind="ExternalOutput").ap()

    # Each stage-1 matmul output S^T[j-chunk, i-block] is [128, IBW] = one
    # PSUM bank; a slab holds SLABW/IBW of them, exp'd by one ACT op.
    cpg = SLABW // IBW       # j-chunks per slab (3)
    groups = []
    c = 0
    while c < NT:
        n = min(cpg, NT - c)
        groups.append((c, n))
        c += n

    with tile.TileContext(nc) as tc, ExitStack() as ctx:
        singles = ctx.enter_context(tc.tile_pool(name="singles", bufs=1))
        nat = ctx.enter_context(tc.tile_pool(name="nat", bufs=2))
        stats = ctx.enter_context(tc.tile_pool(name="stats", bufs=2))
        nrm = ctx.enter_context(tc.tile_pool(name="nrm", bufs=4))
        bigT = ctx.enter_context(tc.tile_pool(name="bigT", bufs=2))
        et_pool = ctx.enter_context(tc.tile_pool(name="et", bufs=3))
        osb_pool = ctx.enter_context(tc.tile_pool(name="osb", bufs=2))
        zr_pool = ctx.enter_context(tc.tile_pool(name="zr", bufs=4))
        out_pool = ctx.enter_context(tc.tile_pool(name="outp", bufs=2))

        ident = singles.tile([128, 128], F32)
        make_identity(nc, ident)
        qs_sb = singles.tile([128, H_PER_CORE], F32)
        nc.sync.dma_start(qs_sb[:], qs_d[:, :])

        QT2, KT2, VX = [], [], []

        # ---------------- PREP: both heads ----------------
        prep_psum = tc.tile_pool(name="ps_t", bufs=2, space="PSUM")
        ps_t = prep_psum.__enter__()
        for h in range(H_PER_CORE):
            qh = q_d[h].rearrange("(p t) d -> p t d", t=NT)
            kh = k_d[h].rearrange("(p t) d -> p t d", t=NT)
            vh = v_d[h].rearrange("(p t) d -> p t d", t=NT)

            q_nat = nat.tile([128, NT, D], F32, tag="qnat")
            k_nat = nat.tile([128, NT, D], F32, tag="knat")
            v_nat = nat.tile([128, NT, D], F32, tag="vnat")
            nc.sync.dma_start(q_nat[:], qh)
            nc.sync.dma_start(k_nat[:], kh)
            nc.sync.dma_start(v_nat[:], vh)

            # sumsq for q (cols 0:NT) and k (cols NT:2NT)
            sq = stats.tile([128, NT, D], F32, tag="sq")
            ss = stats.tile([128, 2 * NT], F32, tag="ss")
            nc.vector.tensor_mul(sq[:], q_nat[:], q_nat[:])
            nc.vector.tensor_reduce(
                ss[:, 0:NT].rearrange("p (t one) -> p t one", one=1), sq[:],
                axis=mybir.AxisListType.X, op=mybir.AluOpType.add)
            nc.vector.tensor_mul(sq[:], k_nat[:], k_nat[:])
            nc.vector.tensor_reduce(
                ss[:, NT:2 * NT].rearrange("p (t one) -> p t one", one=1), sq[:],
                axis=mybir.AxisListType.X, op=mybir.AluOpType.add)
            r = _newton_rsqrt(nc, stats, ss[:], 2 * NT)   # [128, 2NT]
            # fold per-head logit scale into the q side
            nc.vector.tensor_scalar_mul(r[:, 0:NT], r[:, 0:NT], qs_sb[:, h:h + 1])

            qt2 = bigT.tile([128, NT, 128], F32R, tag="qt2")
            kt2 = bigT.tile([128, NPAIR, 128], F32R, tag="kt2")
            vx = bigT.tile([128, NT, D + 1], F32R, tag="vx")
            QT2.append(qt2)
            KT2.append(kt2)
            VX.append(vx)

            # V: round to f32r + ones column (memset can't write f32r, so
            # go through an f32 ones tile and a rounding copy)
            nc.vector.tensor_copy(vx[:, :, 0:D], v_nat[:])
            ones = nrm.tile([128, NT], F32, tag="ones")
            nc.vector.memset(ones[:], 1.0)
            nc.vector.tensor_copy(
                vx[:, :, D:D + 1].rearrange("p t one -> p (t one)"), ones[:])

            # Q: normalize, duplicate into [128, 2, 64], transpose 4 chunks
            # at a time into one [128, 512] psum tile
            for t4 in range(0, NT, 4):
                nt4 = min(4, NT - t4)
                tp = ps_t.tile([128, 4, 128], F32, tag="tp")
                for tt in range(nt4):
                    t = t4 + tt
                    qn2 = nrm.tile([128, 2, D], F32, tag="qn2")
                    nc.vector.tensor_scalar_mul(
                        qn2[:, 0, :], q_nat[:, t, :], r[:, t:t + 1])
                    nc.vector.tensor_scalar_mul(
                        qn2[:, 1, :], q_nat[:, t, :], r[:, t:t + 1])
                    nc.tensor.transpose(
                        tp[:, tt, :], qn2[:].rearrange("p a d -> p (a d)"), ident[:])
                nc.vector.tensor_copy(qt2[:, t4:t4 + nt4, :], tp[:, 0:nt4, :])

            # K: normalize pairs (2a, 2a+1) side by side, transpose
            for a4 in range(0, NPAIR, 4):
                na4 = min(4, NPAIR - a4)
                tp = ps_t.tile([128, 4, 128], F32, tag="tp")
                for aa in range(na4):
                    a = a4 + aa
                    kn2 = nrm.tile([128, 2, D], F32, tag="kn2")
                    nc.vector.tensor_scalar_mul(
                        kn2[:, 0, :], k_nat[:, 2 * a, :], r[:, NT + 2 * a:NT + 2 * a + 1])
                    nc.vector.tensor_scalar_mul(
                        kn2[:, 1, :], k_nat[:, 2 * a + 1, :], r[:, NT + 2 * a + 1:NT + 2 * a + 2])
                    nc.tensor.transpose(
                        tp[:, aa, :], kn2[:].rearrange("p a d -> p (a d)"), ident[:])
                nc.vector.tensor_copy(kt2[:, a4:a4 + na4, :], tp[:, 0:na4, :])
        prep_psum.__exit__(None, None, None)

        # ---------------- MAIN: per head ----------------
        ps_slab = ctx.enter_context(tc.tile_pool(name="ps_slab", bufs=2, space="PSUM"))
        ps_o = ctx.enter_context(tc.tile_pool(name="ps_o", bufs=1, space="PSUM"))
        ps_ot = ctx.enter_context(tc.tile_pool(name="ps_ot", bufs=1, space="PSUM"))
        for h in range(H_PER_CORE):
            qt2, kt2, vx = QT2[h], KT2[h], VX[h]
            out_sb = out_pool.tile([128, NT, D], F32, tag="outsb")
            for ib in range(NIB):
                rhsA = qt2[0:64, CPI * ib:CPI * (ib + 1), :].rearrange("p a b -> p (a b)")
                rhsB = qt2[64:128, CPI * ib:CPI * (ib + 1), :].rearrange("p a b -> p (a b)")
                o_ps = ps_o.tile([65, IBW], F32, tag="ops")
                for (c0, ng) in groups:
                    slab = ps_slab.tile([128, SLABW], F32, tag="slab")
                    for cc in range(ng):
                        c = c0 + cc
                        if c % 2 == 0:
                            nc.tensor.matmul(
                                slab[:, cc * IBW:(cc + 1) * IBW],
                                kt2[0:64, c // 2, :], rhsA,
                                start=True, stop=True, tile_position=(0, 0))
                        else:
                            nc.tensor.matmul(
                                slab[:, cc * IBW:(cc + 1) * IBW],
                                kt2[64:128, c // 2, :], rhsB,
                                start=True, stop=True, tile_position=(64, 0))
                    et = et_pool.tile([128, SLABW], F32R, tag="et")
                    nc.scalar.activation(et[:, 0:ng * IBW], slab[:, 0:ng * IBW],
                                         mybir.ActivationFunctionType.Exp)
                    for cc in range(ng):
                        c = c0 + cc
                        nc.tensor.matmul(
                            o_ps[:], vx[:, c, :], et[:, cc * IBW:(cc + 1) * IBW],
                            start=(c == 0), stop=(c == NT - 1),
                            skip_group_check=True)
                # drain o_ps
                o_sb = osb_pool.tile([65, IBW], F32, tag="osb")
                nc.vector.tensor_copy(o_sb[:], o_ps[:])
                for tchunk in range(CPI):
                    otp = ps_ot.tile([128, D + 1], F32, tag="otp")
                    nc.tensor.transpose(
                        otp[:], o_sb[:, tchunk * 128:(tchunk + 1) * 128],
                        ident[0:65, 0:65])
                    zr = zr_pool.tile([128, 1], F32, tag="zrt")
                    nc.vector.reciprocal_approx_fast(zr[:], otp[:, D:D + 1])
                    nc.vector.tensor_scalar_mul(
                        out_sb[:, CPI * ib + tchunk, :], otp[:, 0:D], zr[:])
            nc.sync.dma_start(
                o_d[h].rearrange("(p t) d -> p t d", t=NT), out_sb[:])

    nc.compile()
    return nc


_NC_CACHE = {}
TRACE = False        # set by test harness for profiling runs
LAST_RESULT = None   # BassKernelResults of the most recent kernel() call


def _get_nc(S):
    if S not in _NC_CACHE:
        _NC_CACHE[S] = build_kernel(S)
    return _NC_CACHE[S]


def kernel(queries, keys, values, logit_scale):
    B, H, S, D_ = queries.shape
    assert B == 1 and D_ == D and H == H_TOTAL
    nc = _get_nc(S)

    # host-side: per-head scale = exp(min(logit_scale, ln sqrt(D)))
    scale = np.exp(np.minimum(np.asarray(logit_scale, np.float32).reshape(H),
                              math.log(math.sqrt(D)))).astype(np.float32)

    in_maps = []
    for c in range(N_CORES):
        h0 = c * H_PER_CORE
        sl = slice(h0, h0 + H_PER_CORE)
        in_maps.append({
            "q": np.ascontiguousarray(queries[0, sl]),
            "k": np.ascontiguousarray(keys[0, sl]),
            "v": np.ascontiguousarray(values[0, sl]),
            "qs": np.ascontiguousarray(
                np.broadcast_to(scale[sl][None, :], (128, H_PER_CORE))),
        })

    res = bass_utils.run_bass_kernel_spmd(
        nc, in_maps, core_ids=list(range(N_CORES)), trace=TRACE)
    global LAST_RESULT
    LAST_RESULT = res

    out = np.empty((B, H, S, D), np.float32)
    for c in range(N_CORES):
        out[0, c * H_PER_CORE:(c + 1) * H_PER_CORE] = res.results[c]["o"]
    return out


# revision 14
# speedup vs baseline: 1.1604x; 1.1604x over previous
"""Cosine attention (B=1, H=16, S=4096, D=64) on 8 trn2 NeuronCores.

Sharding: batch*heads split across cores -> 2 heads per core, full seq per
head (softmax is per-row, no cross-core communication).

Per-head pipeline on a core:
  1. load Q,K,V [4096,64] in "(p t) d" layout -> [128, 32, 64]
  2. row sumsq (ACT Square + DVE reduce), rsqrt via Newton iterations (DVE
     bit-trick, no ACT table switches), fold exp(min(logit_scale, ln 8))
     into the Q-side scale
  3. normalize (one broadcast tensor_tensor per tensor) + PE-transpose to
     d-major float32r: QT2 [128, 32, 128] (partition halves both hold Qt --
     upper half filled by an SBUF->SBUF DMA), KT2 [128, 16, 128] (even
     chunks in partitions 0-63, odd in 64-127)
  4. stage 1: row-packed fp32r matmul pairs S^T[j-chunk, i-block] -> PSUM
     slabs [128, 1536] (ping-pong)
  5. exp on ScalarE straight from PSUM -> E^T tiles (fp16)
  6. stage 2 (fp16): o_ps[65, 512] += Vx[chunk].T @ E^T (Vx has a ones
     column so row 64 accumulates the softmax denominator Z)
  7. drain: copy to SBUF, PE-transpose back to row-major, multiply by
     1/Z (fast reciprocal), one 1 MB DMA out per head.
"""

import math
from contextlib import ExitStack

import numpy as np

import concourse.bass as bass
import concourse.tile as tile
from concourse import bacc, mybir
import concourse.bass_utils as bass_utils
from concourse.masks import make_identity

F32 = mybir.dt.float32
F32R = mybir.dt.float32r
BF16 = mybir.dt.bfloat16
FP16 = mybir.dt.float16
I32 = mybir.dt.int32

N_CORES = 8
H_TOTAL = 16
H_PER_CORE = H_TOTAL // N_CORES
D = 64
IBW = 512          # i-block width (PSUM bank / fp32 moving-operand limit)
SLABW = 1536       # exp slab width (3 PSUM banks)


def _newton_rsqrt(nc, pool, ss, n):
    """rsqrt of ss [128, n] (fp32, positive) via bit-trick seed + 3 Newton
    iterations, all on VectorE. Returns a [128, n] fp32 AP."""
    seed_i = pool.tile([128, n], I32, tag="nw_i")
    # ~(i >> 1)
    nc.vector.tensor_scalar(
        out=seed_i[:], in0=ss.bitcast(I32), scalar1=1, scalar2=-1,
        op0=mybir.AluOpType.logical_shift_right, op1=mybir.AluOpType.bitwise_xor)
    # + (0x5f3759df + 1)  == 0x5f3759df - (i >> 1)
    nc.vector.tensor_scalar(
        out=seed_i[:], in0=seed_i[:], scalar1=0x5F3759E0, scalar2=None,
        op0=mybir.AluOpType.add)
    y = seed_i.bitcast(F32)
    t = pool.tile([128, n], F32, tag="nw_t")
    for _ in range(3):
        nc.vector.tensor_mul(t[:], y, y)            # y*y
        nc.vector.tensor_mul(t[:], t[:], ss)        # x*y*y
        nc.vector.tensor_scalar(                    # 1.5 - 0.5*x*y*y
            out=t[:], in0=t[:], scalar1=-0.5, scalar2=1.5,
            op0=mybir.AluOpType.mult, op1=mybir.AluOpType.add)
        nc.vector.tensor_mul(y, y, t[:])            # y *= t
    return y


def build_kernel(S):
    """Build the 2-head-per-core cosine attention program for seq len S."""
    NT = S // 128            # 128-row tiles per head
    NPAIR = NT // 2
    NIB = S // IBW           # i-blocks
    CPI = IBW // 128         # output chunks per i-block

    nc = bacc.Bacc("TRN2", target_bir_lowering=False, debug=False,
                   enable_asserts=False, num_devices=N_CORES)

    q_d = nc.dram_tensor("q", [H_PER_CORE, S, D], F32, kind="ExternalInput").ap()
    k_d = nc.dram_tensor("k", [H_PER_CORE, S, D], F32, kind="ExternalInput").ap()
    v_d = nc.dram_tensor("v", [H_PER_CORE, S, D], F32, kind="ExternalInput").ap()
    qs_d = nc.dram_tensor("qs", [128, H_PER_CORE], F32, kind="ExternalInput").ap()
    o_d = nc.dram_tensor("o", [H_PER_CORE, S, D], F32, kind="ExternalOutput").ap()

    # Each stage-1 matmul output S^T[j-chunk, i-block] is [128, IBW] = one
    # PSUM bank; a slab holds SLABW/IBW of them, exp'd by one ACT op.
    cpg = SLABW // IBW
    groups = []
    c = 0
    while c < NT:
        n = min(cpg, NT - c)
        groups.append((c, n))
        c += n

    with tile.TileContext(nc) as tc, ExitStack() as ctx:
        singles = ctx.enter_context(tc.tile_pool(name="singles", bufs=1))
        nat = ctx.enter_context(tc.tile_pool(name="nat", bufs=2))
        stats = ctx.enter_context(tc.tile_pool(name="stats", bufs=2))
        bigT = ctx.enter_context(tc.tile_pool(name="bigT", bufs=2))
        et_pool = ctx.enter_context(tc.tile_pool(name="et", bufs=4))
        osb_pool = ctx.enter_context(tc.tile_pool(name="osb", bufs=2))
        zr_pool = ctx.enter_context(tc.tile_pool(name="zr", bufs=4))
        out_pool = ctx.enter_context(tc.tile_pool(name="outp", bufs=2))

        ident = singles.tile([128, 128], F32)
        make_identity(nc, ident)
        qs_sb = singles.tile([128, H_PER_CORE], F32)
        nc.sync.dma_start(qs_sb[:], qs_d[:, :])

        QT2, KT2, VX = [], [], []

        # ---------------- PREP: both heads ----------------
        prep_psum = tc.tile_pool(name="ps_t", bufs=2, space="PSUM")
        ps_t = prep_psum.__enter__()
        for h in range(H_PER_CORE):
            qh = q_d[h].rearrange("(p t) d -> p t d", t=NT)
            kh = k_d[h].rearrange("(p t) d -> p t d", t=NT)
            vh = v_d[h].rearrange("(p t) d -> p t d", t=NT)

            q_nat = nat.tile([128, NT, D], F32, tag="qnat")
            k_nat = nat.tile([128, NT, D], F32, tag="knat")
            v_nat = nat.tile([128, NT, D], F32, tag="vnat")
            nc.sync.dma_start(q_nat[:], qh)
            nc.sync.dma_start(k_nat[:], kh)
            nc.sync.dma_start(v_nat[:], vh)

            # sumsq for q (cols 0:NT) and k (cols NT:2NT); Square on the
            # (prep-idle) ScalarE, reduce on VectorE
            sq = stats.tile([128, NT, D], F32, tag="sq")
            ss = stats.tile([128, 2 * NT], F32, tag="ss")
            nc.scalar.activation(sq[:], q_nat[:],
                                 mybir.ActivationFunctionType.Square)
            nc.vector.tensor_reduce(
                ss[:, 0:NT].rearrange("p (t one) -> p t one", one=1), sq[:],
                axis=mybir.AxisListType.X, op=mybir.AluOpType.add)
            sk = stats.tile([128, NT, D], F32, tag="sk")
            nc.scalar.activation(sk[:], k_nat[:],
                                 mybir.ActivationFunctionType.Square)
            nc.vector.tensor_reduce(
                ss[:, NT:2 * NT].rearrange("p (t one) -> p t one", one=1), sk[:],
                axis=mybir.AxisListType.X, op=mybir.AluOpType.add)
            r = _newton_rsqrt(nc, stats, ss[:], 2 * NT)   # [128, 2NT]
            # fold per-head logit scale into the q side
            nc.vector.tensor_scalar_mul(r[:, 0:NT], r[:, 0:NT], qs_sb[:, h:h + 1])

            # normalize: one broadcast tensor_tensor per tensor
            qn_all = nat.tile([128, NT, D], F32, tag="qnall")
            kn_all = nat.tile([128, NT, D], F32, tag="knall")
            nc.vector.tensor_mul(
                qn_all[:], q_nat[:],
                r[:, 0:NT].rearrange("p (t one) -> p t one", one=1)
                .to_broadcast([128, NT, D]))
            nc.vector.tensor_mul(
                kn_all[:], k_nat[:],
                r[:, NT:2 * NT].rearrange("p (t one) -> p t one", one=1)
                .to_broadcast([128, NT, D]))

            qt2 = bigT.tile([128, NT, 128], F32R, tag="qt2")
            kt2 = bigT.tile([128, NPAIR, 128], F32R, tag="kt2")
            vx = bigT.tile([128, NT, D + 1], FP16, tag="vx")
            QT2.append(qt2)
            KT2.append(kt2)
            VX.append(vx)

            # V: bf16 + ones column (copy on ScalarE, idle during prep)
            nc.scalar.copy(vx[:, :, 0:D], v_nat[:])
            ones = stats.tile([128, NT], F32, tag="ones")
            nc.vector.memset(ones[:], 1.0)
            nc.vector.tensor_copy(
                vx[:, :, D:D + 1].rearrange("p t one -> p (t one)"), ones[:])

            # Q transposes: [128,64] -> [64,128], 4 per PSUM tile, copy to
            # lower half of QT2 on ScalarE, then DMA-duplicate to upper half
            for t4 in range(0, NT, 4):
                nt4 = min(4, NT - t4)
                tp = ps_t.tile([64, 4, 128], F32, tag="tp")
                for tt in range(nt4):
                    t = t4 + tt
                    nc.tensor.transpose(
                        tp[:, tt, :], qn_all[:, t, :], ident[:])
                nc.scalar.copy(qt2[0:64, t4:t4 + nt4, :], tp[:, 0:nt4, :])
            nc.sync.dma_start(qt2[64:128, :, :], qt2[0:64, :, :])

            # K transposes: pairs (2a, 2a+1) side by side -> [128, 128]
            for a4 in range(0, NPAIR, 4):
                na4 = min(4, NPAIR - a4)
                tpk = ps_t.tile([128, 4, 128], F32, tag="tpk")
                for aa in range(na4):
                    a = a4 + aa
                    nc.tensor.transpose(
                        tpk[:, aa, :],
                        kn_all[:, 2 * a:2 * a + 2, :].rearrange("p a d -> p (a d)"),
                        ident[:])
                nc.scalar.copy(kt2[:, a4:a4 + na4, :], tpk[:, 0:na4, :])
        prep_psum.__exit__(None, None, None)

        # ---------------- MAIN: per head ----------------
        ps_slab = ctx.enter_context(tc.tile_pool(name="ps_slab", bufs=2, space="PSUM"))
        ps_o = ctx.enter_context(tc.tile_pool(name="ps_o", bufs=1, space="PSUM"))
        ps_ot = ctx.enter_context(tc.tile_pool(name="ps_ot", bufs=1, space="PSUM"))
        for h in range(H_PER_CORE):
            qt2, kt2, vx = QT2[h], KT2[h], VX[h]
            out_sb = out_pool.tile([128, NT, D], F32, tag="outsb")
            for ib in range(NIB):
                rhsA = qt2[0:64, CPI * ib:CPI * (ib + 1), :].rearrange("p a b -> p (a b)")
                rhsB = qt2[64:128, CPI * ib:CPI * (ib + 1), :].rearrange("p a b -> p (a b)")
                o_ps = ps_o.tile([65, IBW], F32, tag="ops")
                for (c0, ng) in groups:
                    slab = ps_slab.tile([128, SLABW], F32, tag="slab")
                    for cc in range(ng):
                        c = c0 + cc
                        if c % 2 == 0:
                            nc.tensor.matmul(
                                slab[:, cc * IBW:(cc + 1) * IBW],
                                kt2[0:64, c // 2, :], rhsA,
                                start=True, stop=True, tile_position=(0, 0))
                        else:
                            nc.tensor.matmul(
                                slab[:, cc * IBW:(cc + 1) * IBW],
                                kt2[64:128, c // 2, :], rhsB,
                                start=True, stop=True, tile_position=(64, 0))
                    et = et_pool.tile([128, SLABW], FP16, tag="et")
                    nc.scalar.activation(et[:, 0:ng * IBW], slab[:, 0:ng * IBW],
                                         mybir.ActivationFunctionType.Exp)
                    for cc in range(ng):
                        c = c0 + cc
                        nc.tensor.matmul(
                            o_ps[:], vx[:, c, :], et[:, cc * IBW:(cc + 1) * IBW],
                            start=(c == 0), stop=(c == NT - 1),
                            skip_group_check=True)
                # drain o_ps
                o_sb = osb_pool.tile([65, IBW], F32, tag="osb")
                nc.vector.tensor_copy(o_sb[:], o_ps[:])
                for tchunk in range(CPI):
                    otp = ps_ot.tile([128, D + 1], F32, tag="otp")
                    nc.tensor.transpose(
                        otp[:], o_sb[:, tchunk * 128:(tchunk + 1) * 128],
                        ident[0:65, 0:65])
                    zr = zr_pool.tile([128, 1], F32, tag="zrt")
                    nc.vector.reciprocal_approx_fast(zr[:], otp[:, D:D + 1])
                    nc.vector.tensor_scalar_mul(
                        out_sb[:, CPI * ib + tchunk, :], otp[:, 0:D], zr[:])
            nc.sync.dma_start(
                o_d[h].rearrange("(p t) d -> p t d", t=NT), out_sb[:])

    nc.compile()
    return nc


_NC_CACHE = {}
TRACE = False        # set by test harness for profiling runs
LAST_RESULT = None   # BassKernelResults of the most recent kernel() call


def _get_nc(S):
    if S not in _NC_CACHE:
        _NC_CACHE[S] = build_kernel(S)
    return _NC_CACHE[S]


def kernel(queries, keys, values, logit_scale):
    B, H, S, D_ = queries.shape
    assert B == 1 and D_ == D and H == H_TOTAL
    nc = _get_nc(S)

    # host-side: per-head scale = exp(min(logit_scale, ln sqrt(D)))
    scale = np.exp(np.minimum(np.asarray(logit_scale, np.float32).reshape(H),
                              math.log(math.sqrt(D)))).astype(np.float32)

    in_maps = []
    for c in range(N_CORES):
        h0 = c * H_PER_CORE
        sl = slice(h0, h0 + H_PER_CORE)
        in_maps.append({
            "q": np.ascontiguousarray(queries[0, sl]),
            "k": np.ascontiguousarray(keys[0, sl]),
            "v": np.ascontiguousarray(values[0, sl]),
            "qs": np.ascontiguousarray(
                np.broadcast_to(scale[sl][None, :], (128, H_PER_CORE))),
        })

    res = bass_utils.run_bass_kernel_spmd(
        nc, in_maps, core_ids=list(range(N_CORES)), trace=TRACE)
    global LAST_RESULT
    LAST_RESULT = res

    out = np.empty((B, H, S, D), np.float32)
    for c in range(N_CORES):
        out[0, c * H_PER_CORE:(c + 1) * H_PER_CORE] = res.results[c]["o"]
    return out


# revision 15
# speedup vs baseline: 1.2504x; 1.0776x over previous
"""Cosine attention (B=1, H=16, S=4096, D=64) on 8 trn2 NeuronCores.

Sharding: batch*heads split across cores -> 2 heads per core, full seq per
head (softmax is per-row, no cross-core communication).

Per-head pipeline on a core:
  1. load Q,K,V [4096,64] in "(p t) d" layout -> [128, 32, 64]
  2. row sumsq (ACT Square + DVE reduce), rsqrt via Newton iterations (DVE
     bit-trick, no ACT table switches), fold exp(min(logit_scale, ln 8))
     into the Q-side scale
  3. normalize (one broadcast tensor_tensor per tensor) + PE-transpose to
     d-major fp16: QT2 [128, 32, 128] (partition halves both hold Qt --
     upper half filled by an SBUF->SBUF DMA), KT2 [128, 16, 128] (even
     chunks in partitions 0-63, odd in 64-127)
  4. stage 1: row-packed fp16 matmul pairs S^T[j-chunk, i-block] -> PSUM
     slabs [128, 1536] (ping-pong)
  5. exp on ScalarE straight from PSUM -> E^T tiles (fp16)
  6. stage 2 (fp16): o_ps[65, 512] += Vx[chunk].T @ E^T (Vx has a ones
     column so row 64 accumulates the softmax denominator Z)
  7. drain: copy to SBUF, PE-transpose back to row-major, multiply by
     1/Z (fast reciprocal), one 1 MB DMA out per head.
"""

import math
from contextlib import ExitStack

import numpy as np

import concourse.bass as bass
import concourse.tile as tile
from concourse import bacc, mybir
import concourse.bass_utils as bass_utils
from concourse.masks import make_identity

F32 = mybir.dt.float32
F32R = mybir.dt.float32r
BF16 = mybir.dt.bfloat16
FP16 = mybir.dt.float16
I32 = mybir.dt.int32

N_CORES = 8
H_TOTAL = 16
H_PER_CORE = H_TOTAL // N_CORES
D = 64
IBW = 512          # i-block width (PSUM bank / fp32 moving-operand limit)
SLABW = 1536       # exp slab width (3 PSUM banks)


def _newton_rsqrt(nc, pool, ss, n):
    """rsqrt of ss [128, n] (fp32, positive) via bit-trick seed + 3 Newton
    iterations, all on VectorE. Returns a [128, n] fp32 AP."""
    seed_i = pool.tile([128, n], I32, tag="nw_i")
    # ~(i >> 1)
    nc.vector.tensor_scalar(
        out=seed_i[:], in0=ss.bitcast(I32), scalar1=1, scalar2=-1,
        op0=mybir.AluOpType.logical_shift_right, op1=mybir.AluOpType.bitwise_xor)
    # + (0x5f3759df + 1)  == 0x5f3759df - (i >> 1)
    nc.vector.tensor_scalar(
        out=seed_i[:], in0=seed_i[:], scalar1=0x5F3759E0, scalar2=None,
        op0=mybir.AluOpType.add)
    y = seed_i.bitcast(F32)
    t = pool.tile([128, n], F32, tag="nw_t")
    for _ in range(3):
        nc.vector.tensor_mul(t[:], y, y)            # y*y
        nc.vector.tensor_mul(t[:], t[:], ss)        # x*y*y
        nc.vector.tensor_scalar(                    # 1.5 - 0.5*x*y*y
            out=t[:], in0=t[:], scalar1=-0.5, scalar2=1.5,
            op0=mybir.AluOpType.mult, op1=mybir.AluOpType.add)
        nc.vector.tensor_mul(y, y, t[:])            # y *= t
    return y


def build_kernel(S):
    """Build the 2-head-per-core cosine attention program for seq len S."""
    NT = S // 128            # 128-row tiles per head
    NPAIR = NT // 2
    NIB = S // IBW           # i-blocks
    CPI = IBW // 128         # output chunks per i-block

    nc = bacc.Bacc("TRN2", target_bir_lowering=False, debug=False,
                   enable_asserts=False, num_devices=N_CORES)

    q_d = nc.dram_tensor("q", [H_PER_CORE, S, D], F32, kind="ExternalInput").ap()
    k_d = nc.dram_tensor("k", [H_PER_CORE, S, D], F32, kind="ExternalInput").ap()
    v_d = nc.dram_tensor("v", [H_PER_CORE, S, D], F32, kind="ExternalInput").ap()
    qs_d = nc.dram_tensor("qs", [128, H_PER_CORE], F32, kind="ExternalInput").ap()
    o_d = nc.dram_tensor("o", [H_PER_CORE, S, D], F32, kind="ExternalOutput").ap()

    # Each stage-1 matmul output S^T[j-chunk, i-block] is [128, IBW] = one
    # PSUM bank; a slab holds SLABW/IBW of them, exp'd by one ACT op.
    cpg = SLABW // IBW
    groups = []
    c = 0
    while c < NT:
        n = min(cpg, NT - c)
        groups.append((c, n))
        c += n

    with tile.TileContext(nc) as tc, ExitStack() as ctx:
        singles = ctx.enter_context(tc.tile_pool(name="singles", bufs=1))
        nat = ctx.enter_context(tc.tile_pool(name="nat", bufs=2))
        stats = ctx.enter_context(tc.tile_pool(name="stats", bufs=2))
        bigT = ctx.enter_context(tc.tile_pool(name="bigT", bufs=2))
        et_pool = ctx.enter_context(tc.tile_pool(name="et", bufs=4))
        osb_pool = ctx.enter_context(tc.tile_pool(name="osb", bufs=2))
        zr_pool = ctx.enter_context(tc.tile_pool(name="zr", bufs=4))
        out_pool = ctx.enter_context(tc.tile_pool(name="outp", bufs=2))

        ident = singles.tile([128, 128], F32)
        make_identity(nc, ident)
        ident_h = singles.tile([128, 128], FP16)
        nc.vector.tensor_copy(ident_h[:], ident[:])
        qs_sb = singles.tile([128, H_PER_CORE], F32)
        nc.sync.dma_start(qs_sb[:], qs_d[:, :])

        QT2, KT2, VX = [], [], []

        # ---------------- PREP: both heads ----------------
        prep_psum = tc.tile_pool(name="ps_t", bufs=2, space="PSUM")
        ps_t = prep_psum.__enter__()
        for h in range(H_PER_CORE):
            qh = q_d[h].rearrange("(p t) d -> p t d", t=NT)
            kh = k_d[h].rearrange("(p t) d -> p t d", t=NT)
            vh = v_d[h].rearrange("(p t) d -> p t d", t=NT)

            q_nat = nat.tile([128, NT, D], F32, tag="qnat")
            k_nat = nat.tile([128, NT, D], F32, tag="knat")
            v_nat = nat.tile([128, NT, D], F32, tag="vnat")
            nc.sync.dma_start(q_nat[:], qh)
            nc.sync.dma_start(k_nat[:], kh)
            nc.sync.dma_start(v_nat[:], vh)

            # sumsq for q (cols 0:NT) and k (cols NT:2NT); Square on the
            # (prep-idle) ScalarE, reduce on VectorE
            sq = stats.tile([128, NT, D], F32, tag="sq")
            ss = stats.tile([128, 2 * NT], F32, tag="ss")
            nc.vector.tensor_mul(sq[:], q_nat[:], q_nat[:])
            nc.vector.tensor_reduce(
                ss[:, 0:NT].rearrange("p (t one) -> p t one", one=1), sq[:],
                axis=mybir.AxisListType.X, op=mybir.AluOpType.add)
            sk = stats.tile([128, NT, D], F32, tag="sk")
            nc.vector.tensor_mul(sk[:], k_nat[:], k_nat[:])
            nc.vector.tensor_reduce(
                ss[:, NT:2 * NT].rearrange("p (t one) -> p t one", one=1), sk[:],
                axis=mybir.AxisListType.X, op=mybir.AluOpType.add)
            r = _newton_rsqrt(nc, stats, ss[:], 2 * NT)   # [128, 2NT]
            # fold per-head logit scale into the q side
            nc.vector.tensor_scalar_mul(r[:, 0:NT], r[:, 0:NT], qs_sb[:, h:h + 1])

            # normalize: one broadcast tensor_tensor per tensor
            qn_all = nat.tile([128, NT, D], FP16, tag="qnall")
            kn_all = nat.tile([128, NT, D], FP16, tag="knall")
            nc.vector.tensor_mul(
                qn_all[:], q_nat[:],
                r[:, 0:NT].rearrange("p (t one) -> p t one", one=1)
                .to_broadcast([128, NT, D]))
            nc.vector.tensor_mul(
                kn_all[:], k_nat[:],
                r[:, NT:2 * NT].rearrange("p (t one) -> p t one", one=1)
                .to_broadcast([128, NT, D]))

            qt2 = bigT.tile([128, NT, 128], FP16, tag="qt2")
            kt2 = bigT.tile([128, NPAIR, 128], FP16, tag="kt2")
            vx = bigT.tile([128, NT, D + 1], FP16, tag="vx")
            QT2.append(qt2)
            KT2.append(kt2)
            VX.append(vx)

            # V: bf16 + ones column (copy on ScalarE, idle during prep)
            nc.vector.tensor_copy(vx[:, :, 0:D], v_nat[:])
            ones = stats.tile([128, NT], F32, tag="ones")
            nc.vector.memset(ones[:], 1.0)
            nc.vector.tensor_copy(
                vx[:, :, D:D + 1].rearrange("p t one -> p (t one)"), ones[:])

            # Q transposes: [128,64] -> [64,128], 4 per PSUM tile, copy to
            # lower half of QT2 on ScalarE, then DMA-duplicate to upper half
            for t4 in range(0, NT, 4):
                nt4 = min(4, NT - t4)
                tp = ps_t.tile([64, 4, 128], FP16, tag="tp")
                for tt in range(nt4):
                    t = t4 + tt
                    nc.tensor.transpose(
                        tp[:, tt, :], qn_all[:, t, :], ident_h[:])
                nc.vector.tensor_copy(qt2[0:64, t4:t4 + nt4, :], tp[:, 0:nt4, :])
            nc.sync.dma_start(qt2[64:128, :, :], qt2[0:64, :, :])

            # K transposes: pairs (2a, 2a+1) side by side -> [128, 128]
            for a4 in range(0, NPAIR, 4):
                na4 = min(4, NPAIR - a4)
                tpk = ps_t.tile([128, 4, 128], FP16, tag="tpk")
                for aa in range(na4):
                    a = a4 + aa
                    nc.tensor.transpose(
                        tpk[:, aa, :],
                        kn_all[:, 2 * a:2 * a + 2, :].rearrange("p a d -> p (a d)"),
                        ident_h[:])
                nc.vector.tensor_copy(kt2[:, a4:a4 + na4, :], tpk[:, 0:na4, :])
        prep_psum.__exit__(None, None, None)

        # ---------------- MAIN: per head ----------------
        ps_slab = ctx.enter_context(tc.tile_pool(name="ps_slab", bufs=2, space="PSUM"))
        ps_o = ctx.enter_context(tc.tile_pool(name="ps_o", bufs=1, space="PSUM"))
        ps_ot = ctx.enter_context(tc.tile_pool(name="ps_ot", bufs=1, space="PSUM"))
        for h in range(H_PER_CORE):
            qt2, kt2, vx = QT2[h], KT2[h], VX[h]
            out_sb = out_pool.tile([128, NT, D], F32, tag="outsb")
            for ib in range(NIB):
                rhsA = qt2[0:64, CPI * ib:CPI * (ib + 1), :].rearrange("p a b -> p (a b)")
                rhsB = qt2[64:128, CPI * ib:CPI * (ib + 1), :].rearrange("p a b -> p (a b)")
                o_ps = ps_o.tile([65, IBW], F32, tag="ops")
                for (c0, ng) in groups:
                    slab = ps_slab.tile([128, SLABW], F32, tag="slab")
                    for cc in range(ng):
                        c = c0 + cc
                        if c % 2 == 0:
                            nc.tensor.matmul(
                                slab[:, cc * IBW:(cc + 1) * IBW],
                                kt2[0:64, c // 2, :], rhsA,
                                start=True, stop=True, tile_position=(0, 0))
                        else:
                            nc.tensor.matmul(
                                slab[:, cc * IBW:(cc + 1) * IBW],
                                kt2[64:128, c // 2, :], rhsB,
                                start=True, stop=True, tile_position=(64, 0))
                    et = et_pool.tile([128, SLABW], FP16, tag="et")
                    nc.scalar.activation(et[:, 0:ng * IBW], slab[:, 0:ng * IBW],
                                         mybir.ActivationFunctionType.Exp)
                    for cc in range(ng):
                        c = c0 + cc
                        nc.tensor.matmul(
                            o_ps[:], vx[:, c, :], et[:, cc * IBW:(cc + 1) * IBW],
                            start=(c == 0), stop=(c == NT - 1),
                            skip_group_check=True)
                # drain o_ps
                o_sb = osb_pool.tile([65, IBW], F32, tag="osb")
                nc.vector.tensor_copy(o_sb[:], o_ps[:])
                for tchunk in range(CPI):
                    otp = ps_ot.tile([128, D + 1], F32, tag="otp")
                    nc.tensor.transpose(
                        otp[:], o_sb[:, tchunk * 128:(tchunk + 1) * 128],
                        ident[0:65, 0:65])
                    zr = zr_pool.tile([128, 1], F32, tag="zrt")
                    nc.vector.reciprocal_approx_fast(zr[:], otp[:, D:D + 1])
                    nc.vector.tensor_scalar_mul(
                        out_sb[:, CPI * ib + tchunk, :], otp[:, 0:D], zr[:])
            nc.sync.dma_start(
                o_d[h].rearrange("(p t) d -> p t d", t=NT), out_sb[:])

    nc.compile()
    return nc


_NC_CACHE = {}
TRACE = False        # set by test harness for profiling runs
LAST_RESULT = None   # BassKernelResults of the most recent kernel() call


def _get_nc(S):
    if S not in _NC_CACHE:
        _NC_CACHE[S] = build_kernel(S)
    return _NC_CACHE[S]


def kernel(queries, keys, values, logit_scale):
    B, H, S, D_ = queries.shape
    assert B == 1 and D_ == D and H == H_TOTAL
    nc = _get_nc(S)

    # host-side: per-head scale = exp(min(logit_scale, ln sqrt(D)))
    scale = np.exp(np.minimum(np.asarray(logit_scale, np.float32).reshape(H),
                              math.log(math.sqrt(D)))).astype(np.float32)

    in_maps = []
    for c in range(N_CORES):
        h0 = c * H_PER_CORE
        sl = slice(h0, h0 + H_PER_CORE)
        in_maps.append({
            "q": np.ascontiguousarray(queries[0, sl]),
            "k": np.ascontiguousarray(keys[0, sl]),
            "v": np.ascontiguousarray(values[0, sl]),
            "qs": np.ascontiguousarray(
                np.broadcast_to(scale[sl][None, :], (128, H_PER_CORE))),
        })

    res = bass_utils.run_bass_kernel_spmd(
        nc, in_maps, core_ids=list(range(N_CORES)), trace=TRACE)
    global LAST_RESULT
    LAST_RESULT = res

    out = np.empty((B, H, S, D), np.float32)
    for c in range(N_CORES):
        out[0, c * H_PER_CORE:(c + 1) * H_PER_CORE] = res.results[c]["o"]
    return out
